# revision 21
# baseline (speedup 1.0000x reference)
"""MoE FFN (shared SwiGLU expert + top-2-of-16 routed SwiGLU experts) on 8 trn2 cores.

Sharding: expert-parallel (2 routed experts per core), gate replicated
(exact via 6-pass bf16-split matmul), shared expert token-sharded.
Dispatch: on-device routing -> compaction via triangular-matmul prefix
sums -> indirect-DMA scatter of (token-id, weight) records -> indirect
gather of x rows per expert slot -> PE transpose -> fp32r SwiGLU ->
weighted outputs. Host only concatenates shards and gathers each
token's two expert rows by device-computed positions.
"""
import numpy as np
import ml_dtypes

import concourse.bass as bass
import concourse.bacc as bacc
import concourse.mybir as mybir
from concourse.bass_utils import run_bass_kernel_spmd
from concourse.tile import TileContext
from concourse.masks import make_identity, make_upper_triangular

F32 = mybir.dt.float32
F32R = mybir.dt.float32r
BF16 = mybir.dt.bfloat16
I32 = mybir.dt.int32

B, T, D, I, E = 2, 1024, 768, 384, 16
N = B * T            # 2048 tokens
P = 128
NT = N // P          # 16 token tiles
KD = D // P          # 6 contraction chunks
KI = I // P          # 3 inter chunks
C = 384              # capacity per expert
EPC = 2              # experts per core
NCORES = 8
SLOTS = EPC * C      # 768 slots per core
NS = SLOTS // P      # 6 slot chunks per core
TS = N // NCORES     # 256 shared-expert tokens per core
ROUTE_SCALE = 2.5
# 6 bf16-split passes: (x-part, gw-part); 3-way Dekker splits make this
# accurate to ~1e-6, enough to reproduce fp32 top-k selection.
PASSES = [(0, 0), (0, 1), (1, 0), (0, 2), (2, 0), (1, 1)]

_cache = {}


def _split3(a):
    h = a.astype(ml_dtypes.bfloat16)
    r = a - h.astype(np.float32)
    m = r.astype(ml_dtypes.bfloat16)
    l = (r - m.astype(np.float32)).astype(ml_dtypes.bfloat16)
    return (np.ascontiguousarray(h.astype(np.float32)),
            np.ascontiguousarray(m.astype(np.float32)),
            np.ascontiguousarray(l.astype(np.float32)))


def _build():
    nc = bacc.Bacc("TRN2", target_bir_lowering=False, debug=False)

    x_d = nc.dram_tensor("x", [N, D], F32, kind="ExternalInput")
    xph_d = nc.dram_tensor("xph", [D, N], F32R, kind="ExternalInput")
    xpm_d = nc.dram_tensor("xpm", [D, N], F32R, kind="ExternalInput")
    xpl_d = nc.dram_tensor("xpl", [D, N], F32R, kind="ExternalInput")
    gws_d = nc.dram_tensor("gws", [6 * D, E], F32R, kind="ExternalInput")
    xts_d = nc.dram_tensor("xts", [D, TS], F32R, kind="ExternalInput")
    sgT_d = nc.dram_tensor("sgT", [D, I], F32R, kind="ExternalInput")
    suT_d = nc.dram_tensor("suT", [D, I], F32R, kind="ExternalInput")
    sdT_d = nc.dram_tensor("sdT", [I, D], F32R, kind="ExternalInput")
    rgT_d = nc.dram_tensor("rgT", [EPC, D, I], F32R, kind="ExternalInput")
    ruT_d = nc.dram_tensor("ruT", [EPC, D, I], F32R, kind="ExternalInput")
    rdT_d = nc.dram_tensor("rdT", [EPC, I, D], F32R, kind="ExternalInput")
    eoff_d = nc.dram_tensor("eoff", [1, 1], F32, kind="ExternalInput")

    y_d = nc.dram_tensor("y", [SLOTS, D], F32, kind="ExternalOutput")
    sh_d = nc.dram_tensor("sh", [TS, D], F32, kind="ExternalOutput")
    pos_d = nc.dram_tensor("pos", [N, 2], I32, kind="ExternalOutput")
    aux_d = nc.dram_tensor("aux", [1, 1], F32, kind="ExternalOutput")

    idw_d = nc.dram_tensor("idw", [SLOTS, 2], F32)   # internal: [token-id, weight]
    excl_d = nc.dram_tensor("excl", [16, NT], F32)   # internal: exclusive tile offsets

    with TileContext(nc) as tc, \
         tc.tile_pool(name="wp", bufs=1) as wp, \
         tc.tile_pool(name="big", bufs=1) as big, \
         tc.tile_pool(name="xp", bufs=2) as xp, \
         tc.tile_pool(name="wk", bufs=2) as wk, \
         tc.tile_pool(name="ps", bufs=2, space="PSUM") as ps:

        # ---------- constants / persistent weights ----------
        ident = wp.tile([P, P], F32)
        make_identity(nc, ident[:])
        sut = wp.tile([P, P], F32)
        make_upper_triangular(nc, sut[:], val=1.0, diag=False)
        ones1 = wp.tile([1, P], F32)
        nc.vector.memset(ones1[:], 1.0)
        onesP = wp.tile([P, 1], F32)
        nc.vector.memset(onesP[:], 1.0)
        iota16 = wp.tile([P, E], I32)
        nc.gpsimd.iota(iota16[:], pattern=[[1, E]], base=0, channel_multiplier=0)
        iota16f = wp.tile([P, E], F32)
        nc.vector.tensor_copy(iota16f[:], iota16[:])
        tokid = wp.tile([P, NT], I32)   # tokid[p, t] = t*128 + p
        nc.gpsimd.iota(tokid[:], pattern=[[P, NT]], base=0, channel_multiplier=1)
        tokidf = wp.tile([P, NT], F32)
        nc.vector.tensor_copy(tokidf[:], tokid[:])

        gwt = wp.tile([P, 36 * E], F32R)
        nc.sync.dma_start(out=gwt[:].rearrange("p (a e) -> p a e", a=36),
                          in_=gws_d[:].rearrange("(a d) e -> d a e", d=P))

        sgT = [wp.tile([P, I], F32R, tag=f"sgT{k}", name=f"sgT{k}") for k in range(KD)]
        suT = [wp.tile([P, I], F32R, tag=f"suT{k}", name=f"suT{k}") for k in range(KD)]
        for k in range(KD):
            nc.sync.dma_start(out=sgT[k][:], in_=sgT_d[k * P:(k + 1) * P, :])
            nc.sync.dma_start(out=suT[k][:], in_=suT_d[k * P:(k + 1) * P, :])
        sdT = [wp.tile([P, D], F32R, tag=f"sdT{i}", name=f"sdT{i}") for i in range(KI)]
        for i in range(KI):
            nc.sync.dma_start(out=sdT[i][:], in_=sdT_d[i * P:(i + 1) * P, :])
        rgT = [[wp.tile([P, I], F32R, tag=f"rgT{e}_{k}", name=f"rgT{e}_{k}") for k in range(KD)] for e in range(EPC)]
        ruT = [[wp.tile([P, I], F32R, tag=f"ruT{e}_{k}", name=f"ruT{e}_{k}") for k in range(KD)] for e in range(EPC)]
        rdT = [[wp.tile([P, D], F32R, tag=f"rdT{e}_{i}", name=f"rdT{e}_{i}") for i in range(KI)] for e in range(EPC)]
        for e in range(EPC):
            for k in range(KD):
                nc.sync.dma_start(out=rgT[e][k][:], in_=rgT_d[e, k * P:(k + 1) * P, :])
                nc.sync.dma_start(out=ruT[e][k][:], in_=ruT_d[e, k * P:(k + 1) * P, :])
            for i in range(KI):
                nc.sync.dma_start(out=rdT[e][i][:], in_=rdT_d[e, i * P:(i + 1) * P, :])

        eoffs = wp.tile([1, 1], F32)
        nc.sync.dma_start(out=eoffs[:], in_=eoff_d[:])
        ebp = ps.tile([P, 1], F32, tag="ps")
        nc.tensor.matmul(ebp[:], ones1[:], eoffs[:], start=True, stop=True)
        ebc = wp.tile([P, 1], F32)
        nc.vector.tensor_copy(ebc[:], ebp[:])

        # ---------- exact gate: 6 bf16-split passes ----------
        logT = big.tile([P, N], F32)    # rows 0:16 hold logits.T (scaled)
        for n in range(4):
            gpsn = ps.tile([16, 512], F32, tag="gate", name="gpsn", bufs=2)
            for k in range(KD):
                xparts = []
                for pi2, src in enumerate((xph_d, xpm_d, xpl_d)):
                    xt_ = xp.tile([P, 512], F32R, tag=f"xgate{pi2}", name=f"xgate{pi2}")
                    nc.sync.dma_start(out=xt_[:], in_=src[k * P:(k + 1) * P, n * 512:(n + 1) * 512])
                    xparts.append(xt_)
                for pi, (xi, gi) in enumerate(PASSES):
                    lidx = pi * 6 + k
                    nc.tensor.matmul(
                        gpsn[:],
                        gwt[:, lidx * E:(lidx + 1) * E],
                        xparts[xi][:],
                        start=(k == 0 and pi == 0), stop=(k == KD - 1 and pi == len(PASSES) - 1),
                    )
            nc.vector.tensor_scalar_mul(logT[0:16, n * 512:(n + 1) * 512], gpsn[:], ROUTE_SCALE)

        # ---------- routing per token tile ----------
        eq1b = big.tile([P, NT * E], F32)
        eq2b = big.tile([P, NT * E], F32)
        maskb = big.tile([P, NT * E], F32)
        w1b = big.tile([P, NT], F32)
        w2b = big.tile([P, NT], F32)
        i1b = big.tile([P, NT], F32)
        i2b = big.tile([P, NT], F32)
        colsT = big.tile([16, NT], F32)
        sacc = big.tile([P, E], F32)   # softmax score accumulator
        nc.vector.memset(sacc[:], 0.0)

        for t in range(NT):
            ltp = ps.tile([P, P], F32, tag="ps")
            nc.tensor.transpose(out=ltp[:], in_=logT[:, t * P:(t + 1) * P], identity=ident[:])
            lt = wk.tile([P, E], F32, tag="lt")
            nc.vector.tensor_copy(lt[:], ltp[:, 0:E])

            top = wk.tile([P, 8], F32, tag="top")
            tix = wk.tile([P, 8], mybir.dt.uint32, tag="tix")
            nc.vector.max(out=top[:], in_=lt[:])
            nc.vector.max_index(out=tix[:], in_max=top[:], in_values=lt[:])
            nc.vector.tensor_copy(i1b[:, t:t + 1], tix[:, 0:1])
            nc.vector.tensor_copy(i2b[:, t:t + 1], tix[:, 1:2])

            dif = wk.tile([P, 1], F32, tag="dif")
            nc.vector.tensor_tensor(out=dif[:], in0=top[:, 0:1], in1=top[:, 1:2], op=mybir.AluOpType.subtract)
            nc.scalar.activation(w1b[:, t:t + 1], dif[:], mybir.ActivationFunctionType.Sigmoid)
            nc.scalar.activation(w2b[:, t:t + 1], dif[:], mybir.ActivationFunctionType.Sigmoid, scale=-1.0)

            eq1 = eq1b[:, t * E:(t + 1) * E]
            eq2 = eq2b[:, t * E:(t + 1) * E]
            msk = maskb[:, t * E:(t + 1) * E]
            nc.vector.tensor_scalar(eq1, iota16f[:], i1b[:, t:t + 1], None, op0=mybir.AluOpType.is_equal)
            nc.vector.tensor_scalar(eq2, iota16f[:], i2b[:, t:t + 1], None, op0=mybir.AluOpType.is_equal)
            nc.vector.tensor_tensor(out=msk, in0=eq1, in1=eq2, op=mybir.AluOpType.add)

            csp = ps.tile([16, 1], F32, tag="ps")
            nc.tensor.matmul(csp[:], msk, onesP[:], start=True, stop=True)
            nc.vector.tensor_copy(colsT[:, t:t + 1], csp[:])

            # softmax scores for aux loss
            mx = wk.tile([P, 1], F32, tag="mx")
            nc.vector.reduce_max(mx[:], lt[:], axis=mybir.AxisListType.X)
            nmx = wk.tile([P, 1], F32, tag="nmx")
            nc.vector.tensor_scalar_mul(nmx[:], mx[:], -1.0)
            ex = wk.tile([P, E], F32, tag="ex")
            nc.scalar.activation(ex[:], lt[:], mybir.ActivationFunctionType.Exp, bias=nmx[:])
            sm = wk.tile([P, 1], F32, tag="sm")
            nc.vector.reduce_sum(sm[:], ex[:], axis=mybir.AxisListType.X)
            rs = wk.tile([P, 1], F32, tag="rs")
            nc.vector.reciprocal(rs[:], sm[:])
            sc = wk.tile([P, E], F32, tag="sc")
            nc.vector.tensor_scalar(sc[:], ex[:], rs[:], None, op0=mybir.AluOpType.mult)
            nc.vector.tensor_tensor(out=sacc[:], in0=sacc[:], in1=sc[:], op=mybir.AluOpType.add)

        # ---------- capacity offsets: exclusive scan over tiles ----------
        incl = big.tile([16, NT], F32)
        zer = wk.tile([16, NT], F32, tag="zer")
        nc.vector.memset(zer[:], 0.0)
        nc.vector.tensor_tensor_scan(incl[:], colsT[:], zer[:], 0.0,
                                     op0=mybir.AluOpType.add, op1=mybir.AluOpType.add)
        excl = big.tile([16, NT], F32)
        nc.vector.tensor_tensor(out=excl[:], in0=incl[:], in1=colsT[:], op=mybir.AluOpType.subtract)
        nc.sync.dma_start(out=excl_d[:], in_=excl[:])

        # ---------- positions + dispatch records ----------
        pos1 = big.tile([P, NT], F32)
        pos2 = big.tile([P, NT], F32)
        for t in range(NT):
            otile = wk.tile([1, E], F32, tag="otile", name="otile")
            nc.sync.dma_start(out=otile[:], in_=excl_d[:, t:t + 1].rearrange("a b -> b a"))
            rp = ps.tile([P, E], F32, tag="ps")
            nc.tensor.matmul(rp[:], sut[:], maskb[:, t * E:(t + 1) * E], start=True, stop=False)
            nc.tensor.matmul(rp[:], ones1[:], otile[:], start=False, stop=True)
            for (eqb, ib, posb) in ((eq1b, i1b, pos1), (eq2b, i2b, pos2)):
                tmp = wk.tile([P, E], F32, tag="tmp")
                nc.vector.tensor_tensor(out=tmp[:], in0=rp[:], in1=eqb[:, t * E:(t + 1) * E], op=mybir.AluOpType.mult)
                slot = wk.tile([P, 1], F32, tag="slot")
                nc.vector.reduce_sum(slot[:], tmp[:], axis=mybir.AxisListType.X)
                nc.vector.tensor_scalar(posb[:, t:t + 1], ib[:, t:t + 1], float(C), slot[:],
                                        op0=mybir.AluOpType.mult, op1=mybir.AluOpType.add)

        posI = big.tile([P, NT, 2], I32)
        nc.vector.tensor_copy(posI[:, :, 0], pos1[:])
        nc.vector.tensor_copy(posI[:, :, 1], pos2[:])
        nc.sync.dma_start(out=pos_d[:].rearrange("(t p) k -> p t k", p=P), in_=posI[:])

        # local offsets; negatives -> huge so bounds check drops them
        locs = []
        for _ki, posb in enumerate((pos1, pos2)):
            loc = big.tile([P, NT], F32, name=f"loc{_ki}", tag=f"loc{_ki}")
            nc.vector.tensor_scalar(loc[:], posb[:], ebc[:], None, op0=mybir.AluOpType.subtract)
            neg = wk.tile([P, NT], F32, tag="neg")
            nc.vector.tensor_scalar(neg[:], loc[:], 0.0, 1.0e6, op0=mybir.AluOpType.is_lt, op1=mybir.AluOpType.mult)
            nc.vector.tensor_tensor(out=loc[:], in0=loc[:], in1=neg[:], op=mybir.AluOpType.add)
            locI = big.tile([P, NT], I32, name=f"locI{_ki}", tag=f"locI{_ki}")
            nc.vector.tensor_copy(locI[:], loc[:])
            locs.append(locI)

        recall = big.tile([P, NT, 2, 2], F32)   # [p, t, k, (id, w)]
        for ki, wb in ((0, w1b), (1, w2b)):
            nc.vector.tensor_copy(recall[:, :, ki, 0], tokidf[:])
            nc.vector.tensor_copy(recall[:, :, ki, 1], wb[:])
        for t in range(NT):
            for ki, locI in enumerate(locs):
                nc.gpsimd.indirect_dma_start(
                    out=idw_d[:], out_offset=bass.IndirectOffsetOnAxis(ap=locI[:, t:t + 1], axis=0),
                    in_=recall[:, t, ki, :], in_offset=None, bounds_check=SLOTS - 1, oob_is_err=False)

        # ---------- aux loss ----------
        pf = ps.tile([16, 1], F32, tag="ps")
        nc.tensor.matmul(pf[:], sacc[:], onesP[:], start=True, stop=True)
        pfs = wk.tile([16, 1], F32, tag="pfs")
        nc.vector.tensor_copy(pfs[:], pf[:])
        prod = wk.tile([16, 1], F32, tag="prod")
        nc.vector.tensor_tensor(out=prod[:], in0=pfs[:], in1=incl[:, NT - 1:NT], op=mybir.AluOpType.mult)
        axp = ps.tile([1, 1], F32, tag="ps")
        nc.tensor.matmul(axp[:], prod[:], onesP[0:16, 0:1], start=True, stop=True)
        axs = wk.tile([1, 1], F32, tag="axs")
        nc.vector.tensor_scalar_mul(axs[:], axp[:], float(E) / (N * N))
        nc.sync.dma_start(out=aux_d[:], in_=axs[:])

        tc.no_sync_barrier()

        # ---------- shared expert on this core's token shard ----------
        xs = [wp.tile([P, TS], F32R, tag=f"xs{k}", name=f"xs{k}") for k in range(KD)]
        for k in range(KD):
            nc.sync.dma_start(out=xs[k][:], in_=xts_d[k * P:(k + 1) * P, :])
        hs = [wp.tile([P, TS], F32R, tag=f"hs{i}", name=f"hs{i}") for i in range(KI)]
        for i in range(KI):
            gp = ps.tile([P, TS], F32, tag="gpup", bufs=2, name="gp")
            up = ps.tile([P, TS], F32, tag="gpup", bufs=2, name="up")
            for k in range(KD):
                nc.tensor.matmul(gp[:], sgT[k][:, i * P:(i + 1) * P], xs[k][:],
                                 start=(k == 0), stop=(k == KD - 1))
            for k in range(KD):
                nc.tensor.matmul(up[:], suT[k][:, i * P:(i + 1) * P], xs[k][:],
                                 start=(k == 0), stop=(k == KD - 1))
            sg = wk.tile([P, TS], F32, tag="sgs")
            nc.scalar.activation(sg[:], gp[:], mybir.ActivationFunctionType.Sigmoid)
            sil = wk.tile([P, TS], F32, tag="sils")
            nc.vector.tensor_tensor(out=sil[:], in0=sg[:], in1=gp[:], op=mybir.AluOpType.mult)
            nc.vector.tensor_tensor(out=hs[i][:], in0=sil[:], in1=up[:], op=mybir.AluOpType.mult)
        for sl in range(TS // P):
            shs = wk.tile([P, D], F32, tag="shs")
            for nh in range(2):
                sp = ps.tile([P, I], F32, tag="ps")
                for i in range(KI):
                    nc.tensor.matmul(sp[:], hs[i][:, sl * P:(sl + 1) * P], sdT[i][:, nh * I:(nh + 1) * I],
                                     start=(i == 0), stop=(i == KI - 1))
                nc.vector.tensor_copy(shs[:, nh * I:(nh + 1) * I], sp[:])
            nc.sync.dma_start(out=sh_d[sl * P:(sl + 1) * P, :], in_=shs[:])


        tc.no_sync_barrier()

        # ---------- dispatch gather + transpose ----------
        XT = [[wp.tile([P, C], F32R, tag=f"XT{e}_{k}", name=f"XT{e}_{k}") for k in range(KD)] for e in range(EPC)]
        wslot = [wp.tile([P, 1], F32, tag=f"ws{s}", name=f"ws{s}") for s in range(NS)]
        for s in range(NS):
            e, sl = s // KI, s % KI
            idw = wk.tile([P, 2], F32, tag="idw")
            nc.sync.dma_start(out=idw[:], in_=idw_d[s * P:(s + 1) * P, :])
            nc.vector.tensor_copy(wslot[s][:], idw[:, 1:2])
            idsi = wk.tile([P, 1], I32, tag="idsi")
            nc.vector.tensor_copy(idsi[:], idw[:, 0:1])
            xg = wk.tile([P, D], F32, tag="xg")
            nc.gpsimd.indirect_dma_start(
                out=xg[:], out_offset=None, in_=x_d[:],
                in_offset=bass.IndirectOffsetOnAxis(ap=idsi[:, :1], axis=0),
                bounds_check=N - 1, oob_is_err=False)
            for k in range(KD):
                tp = ps.tile([P, P], F32, tag="tp", bufs=2)
                nc.tensor.transpose(out=tp[:], in_=xg[:, k * P:(k + 1) * P], identity=ident[:])
                nc.vector.tensor_copy(XT[e][k][:, sl * P:(sl + 1) * P], tp[:])

        tc.no_sync_barrier()

        # ---------- routed experts FFN (fp32r) ----------
        for e in range(EPC):
            hh = [wp.tile([P, C], F32R, tag=f"hh{e}_{i}", name=f"hh{e}_{i}") for i in range(KI)]
            for i in range(KI):
                gp = ps.tile([P, C], F32, tag="gpup", bufs=2, name="gp")
                up = ps.tile([P, C], F32, tag="gpup", bufs=2, name="up")
                for k in range(KD):
                    nc.tensor.matmul(gp[:], rgT[e][k][:, i * P:(i + 1) * P], XT[e][k][:],
                                     start=(k == 0), stop=(k == KD - 1))
                for k in range(KD):
                    nc.tensor.matmul(up[:], ruT[e][k][:, i * P:(i + 1) * P], XT[e][k][:],
                                     start=(k == 0), stop=(k == KD - 1))
                sg = wk.tile([P, C], F32, tag="sg")
                nc.scalar.activation(sg[:], gp[:], mybir.ActivationFunctionType.Sigmoid)
                sil = wk.tile([P, C], F32, tag="sil")
                nc.vector.tensor_tensor(out=sil[:], in0=sg[:], in1=gp[:], op=mybir.AluOpType.mult)
                nc.vector.tensor_tensor(out=hh[i][:], in0=sil[:], in1=up[:], op=mybir.AluOpType.mult)
            for sl in range(KI):
                ysb = wk.tile([P, D], F32, tag="ysb")
                for nh in range(2):
                    yp = ps.tile([P, I], F32, tag="ps")
                    for i in range(KI):
                        nc.tensor.matmul(yp[:], hh[i][:, sl * P:(sl + 1) * P], rdT[e][i][:, nh * I:(nh + 1) * I],
                                         start=(i == 0), stop=(i == KI - 1))
                    nc.vector.tensor_scalar(ysb[:, nh * I:(nh + 1) * I], yp[:], wslot[e * KI + sl][:], None,
                                            op0=mybir.AluOpType.mult)
                nc.sync.dma_start(out=y_d[(e * KI + sl) * P:(e * KI + sl + 1) * P, :], in_=ysb[:])

    nc.compile()
    return nc


def kernel(x, gate_w, sg_w, su_w, sd_w, rg_w, ru_w, rd_w):
    x = np.ascontiguousarray(np.asarray(x, dtype=np.float32))
    flat = x.reshape(N, D)
    xT = np.ascontiguousarray(flat.T)
    xh, xm, xl = _split3(xT)
    gwT = np.ascontiguousarray(np.asarray(gate_w, np.float32).T)   # (D, E)
    gh, gm, gl = _split3(gwT)
    gparts = (gh, gm, gl)
    gws = np.concatenate([np.asarray(gparts[gi], dtype=np.float32) for (_, gi) in PASSES], axis=0)

    sgT = np.ascontiguousarray(np.asarray(sg_w, np.float32).T)
    suT = np.ascontiguousarray(np.asarray(su_w, np.float32).T)
    sdT = np.ascontiguousarray(np.asarray(sd_w, np.float32).T)
    rgT = np.ascontiguousarray(np.asarray(rg_w, np.float32).transpose(0, 2, 1))
    ruT = np.ascontiguousarray(np.asarray(ru_w, np.float32).transpose(0, 2, 1))
    rdT = np.ascontiguousarray(np.asarray(rd_w, np.float32).transpose(0, 2, 1))

    if "nc" not in _cache:
        _cache["nc"] = _build()
    nc = _cache["nc"]

    in_maps = []
    for c in range(NCORES):
        in_maps.append({
            "x": flat,
            "xph": xh, "xpm": xm, "xpl": xl,
            "gws": gws,
            "xts": np.ascontiguousarray(xT[:, c * TS:(c + 1) * TS]),
            "sgT": sgT, "suT": suT, "sdT": sdT,
            "rgT": rgT[c * EPC:(c + 1) * EPC],
            "ruT": ruT[c * EPC:(c + 1) * EPC],
            "rdT": rdT[c * EPC:(c + 1) * EPC],
            "eoff": np.full((1, 1), c * EPC * C, np.float32),
        })
    res = run_bass_kernel_spmd(nc, in_maps, core_ids=list(range(NCORES))).results

    y_all = np.concatenate([r["y"] for r in res], axis=0)          # (E*C, D)
    sh_all = np.concatenate([r["sh"] for r in res], axis=0)        # (N, D)
    pos = res[0]["pos"]                                            # (N, 2)
    out = sh_all + y_all[pos[:, 0]] + y_all[pos[:, 1]]
    aux = np.float32(res[0]["aux"][0, 0])
    return out.reshape(B, T, D), np.asarray(aux, dtype=np.float32)


# revision 22
# speedup vs baseline: 1.2575x; 1.2575x over previous
"""MoE FFN (shared SwiGLU expert + top-2-of-16 routed SwiGLU experts) on 8 trn2 cores.

Sharding: expert-parallel (2 routed experts per core), gate replicated
(exact via 6-pass bf16-split matmul), shared expert token-sharded.
Dispatch: on-device routing -> compaction via triangular-matmul prefix
sums -> indirect-DMA scatter of (token-id, weight) records -> indirect
gather of x rows per expert slot -> PE transpose -> fp32r SwiGLU ->
weighted outputs. Host only concatenates shards and gathers each
token's two expert rows by device-computed positions.
"""
import numpy as np
import ml_dtypes

import concourse.bass as bass
import concourse.bacc as bacc
import concourse.mybir as mybir
from concourse.bass_utils import run_bass_kernel_spmd
from concourse.tile import TileContext
from concourse.masks import make_identity, make_upper_triangular

F32 = mybir.dt.float32
F32R = mybir.dt.float32r
BF16 = mybir.dt.bfloat16
I32 = mybir.dt.int32

B, T, D, I, E = 2, 1024, 768, 384, 16
N = B * T            # 2048 tokens
P = 128
NT = N // P          # 16 token tiles
KD = D // P          # 6 contraction chunks
KI = I // P          # 3 inter chunks
C = 384              # capacity per expert
EPC = 2              # experts per core
NCORES = 8
SLOTS = EPC * C      # 768 slots per core
NS = SLOTS // P      # 6 slot chunks per core
TS = N // NCORES     # 256 shared-expert tokens per core
ROUTE_SCALE = 2.5
# 5 split passes: x-side = (xh = bf16(x) as fp32r-exact, xr = x - xh exact fp32,
# truncated by fp32r to ~12 bits -> error ~2^-21); g-side = 3-way bf16 Dekker
# split (exact). Total gate error ~1e-6, enough to reproduce fp32 top-k.
PASSES = [(0, 0), (0, 1), (1, 0), (0, 2), (1, 1)]
NPASS = len(PASSES)

_cache = {}


def _split3(a):
    h = a.astype(ml_dtypes.bfloat16)
    r = a - h.astype(np.float32)
    m = r.astype(ml_dtypes.bfloat16)
    l = (r - m.astype(np.float32)).astype(ml_dtypes.bfloat16)
    return (np.ascontiguousarray(h.astype(np.float32)),
            np.ascontiguousarray(m.astype(np.float32)),
            np.ascontiguousarray(l.astype(np.float32)))


def _build():
    nc = bacc.Bacc("TRN2", target_bir_lowering=False, debug=False)

    x_d = nc.dram_tensor("x", [N, D], F32, kind="ExternalInput")
    xph_d = nc.dram_tensor("xph", [D, N], F32R, kind="ExternalInput")
    xpm_d = nc.dram_tensor("xpm", [D, N], F32R, kind="ExternalInput")
    gws_d = nc.dram_tensor("gws", [NPASS * D, E], F32R, kind="ExternalInput")
    xts_d = nc.dram_tensor("xts", [D, TS], F32R, kind="ExternalInput")
    sgT_d = nc.dram_tensor("sgT", [D, I], F32R, kind="ExternalInput")
    suT_d = nc.dram_tensor("suT", [D, I], F32R, kind="ExternalInput")
    sdT_d = nc.dram_tensor("sdT", [I, D], F32R, kind="ExternalInput")
    rgT_d = nc.dram_tensor("rgT", [EPC, D, I], F32R, kind="ExternalInput")
    ruT_d = nc.dram_tensor("ruT", [EPC, D, I], F32R, kind="ExternalInput")
    rdT_d = nc.dram_tensor("rdT", [EPC, I, D], F32R, kind="ExternalInput")
    eoff_d = nc.dram_tensor("eoff", [1, 1], F32, kind="ExternalInput")

    y_d = nc.dram_tensor("y", [SLOTS, D], F32, kind="ExternalOutput")
    sh_d = nc.dram_tensor("sh", [TS, D], F32, kind="ExternalOutput")
    pos_d = nc.dram_tensor("pos", [N, 2], I32, kind="ExternalOutput")
    aux_d = nc.dram_tensor("aux", [1, 1], F32, kind="ExternalOutput")

    idw_d = nc.dram_tensor("idw", [SLOTS, 2], F32)   # internal: [token-id, weight]
    excl_d = nc.dram_tensor("excl", [16, NT], F32)   # internal: exclusive tile offsets

    with TileContext(nc) as tc, \
         tc.tile_pool(name="wp", bufs=1) as wp, \
         tc.tile_pool(name="big", bufs=1) as big, \
         tc.tile_pool(name="xp", bufs=2) as xp, \
         tc.tile_pool(name="wk", bufs=2) as wk, \
         tc.tile_pool(name="ps", bufs=2, space="PSUM") as ps:

        # ---------- constants / persistent weights ----------
        ident = wp.tile([P, P], F32)
        make_identity(nc, ident[:])
        sut = wp.tile([P, P], F32)
        make_upper_triangular(nc, sut[:], val=1.0, diag=False)
        ones1 = wp.tile([1, P], F32)
        nc.vector.memset(ones1[:], 1.0)
        onesP = wp.tile([P, 1], F32)
        nc.vector.memset(onesP[:], 1.0)
        iota16 = wp.tile([P, E], I32)
        nc.gpsimd.iota(iota16[:], pattern=[[1, E]], base=0, channel_multiplier=0)
        iota16f = wp.tile([P, E], F32)
        nc.vector.tensor_copy(iota16f[:], iota16[:])
        tokid = wp.tile([P, NT], I32)   # tokid[p, t] = t*128 + p
        nc.gpsimd.iota(tokid[:], pattern=[[P, NT]], base=0, channel_multiplier=1)
        tokidf = wp.tile([P, NT], F32)
        nc.vector.tensor_copy(tokidf[:], tokid[:])

        gwt = wp.tile([P, NPASS * KD * E], F32R)
        nc.sync.dma_start(out=gwt[:].rearrange("p (a e) -> p a e", a=NPASS * KD),
                          in_=gws_d[:].rearrange("(a d) e -> d a e", d=P))

        sgT = [wp.tile([P, I], F32R, tag=f"sgT{k}", name=f"sgT{k}") for k in range(KD)]
        suT = [wp.tile([P, I], F32R, tag=f"suT{k}", name=f"suT{k}") for k in range(KD)]
        for k in range(KD):
            nc.sync.dma_start(out=sgT[k][:], in_=sgT_d[k * P:(k + 1) * P, :])
            nc.sync.dma_start(out=suT[k][:], in_=suT_d[k * P:(k + 1) * P, :])
        sdT = [wp.tile([P, D], F32R, tag=f"sdT{i}", name=f"sdT{i}") for i in range(KI)]
        for i in range(KI):
            nc.sync.dma_start(out=sdT[i][:], in_=sdT_d[i * P:(i + 1) * P, :])
        rgT = [[wp.tile([P, I], F32R, tag=f"rgT{e}_{k}", name=f"rgT{e}_{k}") for k in range(KD)] for e in range(EPC)]
        ruT = [[wp.tile([P, I], F32R, tag=f"ruT{e}_{k}", name=f"ruT{e}_{k}") for k in range(KD)] for e in range(EPC)]
        rdT = [[wp.tile([P, D], F32R, tag=f"rdT{e}_{i}", name=f"rdT{e}_{i}") for i in range(KI)] for e in range(EPC)]
        for e in range(EPC):
            for k in range(KD):
                nc.sync.dma_start(out=rgT[e][k][:], in_=rgT_d[e, k * P:(k + 1) * P, :])
                nc.sync.dma_start(out=ruT[e][k][:], in_=ruT_d[e, k * P:(k + 1) * P, :])
            for i in range(KI):
                nc.sync.dma_start(out=rdT[e][i][:], in_=rdT_d[e, i * P:(i + 1) * P, :])

        eoffs = wp.tile([1, 1], F32)
        nc.sync.dma_start(out=eoffs[:], in_=eoff_d[:])
        ebp = ps.tile([P, 1], F32, tag="ps")
        nc.tensor.matmul(ebp[:], ones1[:], eoffs[:], start=True, stop=True)
        ebc = wp.tile([P, 1], F32)
        nc.vector.tensor_copy(ebc[:], ebp[:])

        # ---------- exact gate: 6 bf16-split passes ----------
        logT = big.tile([P, N], F32)    # rows 0:16 hold logits.T (scaled)
        for n in range(4):
            gpsn = ps.tile([16, 512], F32, tag="gate", name="gpsn", bufs=2)
            for k in range(KD):
                xparts = []
                for pi2, src in enumerate((xph_d, xpm_d)):
                    xt_ = xp.tile([P, 512], F32R, tag=f"xgate{pi2}", name=f"xgate{pi2}")
                    nc.sync.dma_start(out=xt_[:], in_=src[k * P:(k + 1) * P, n * 512:(n + 1) * 512])
                    xparts.append(xt_)
                for pi, (xi, gi) in enumerate(PASSES):
                    lidx = pi * 6 + k
                    nc.tensor.matmul(
                        gpsn[:],
                        gwt[:, lidx * E:(lidx + 1) * E],
                        xparts[xi][:],
                        start=(k == 0 and pi == 0), stop=(k == KD - 1 and pi == len(PASSES) - 1),
                    )
            nc.vector.tensor_scalar_mul(logT[0:16, n * 512:(n + 1) * 512], gpsn[:], ROUTE_SCALE)

        # ---------- routing per token tile ----------
        eq1b = big.tile([P, NT * E], F32)
        eq2b = big.tile([P, NT * E], F32)
        maskb = big.tile([P, NT * E], F32)
        w1b = big.tile([P, NT], F32)
        w2b = big.tile([P, NT], F32)
        i1b = big.tile([P, NT], F32)
        i2b = big.tile([P, NT], F32)
        colsT = big.tile([16, NT], F32)
        sacc = big.tile([P, E], F32)   # softmax score accumulator
        nc.vector.memset(sacc[:], 0.0)

        for t in range(NT):
            ltp = ps.tile([P, P], F32, tag="ps")
            nc.tensor.transpose(out=ltp[:], in_=logT[:, t * P:(t + 1) * P], identity=ident[:])
            lt = wk.tile([P, E], F32, tag="lt")
            nc.vector.tensor_copy(lt[:], ltp[:, 0:E])

            top = wk.tile([P, 8], F32, tag="top")
            tix = wk.tile([P, 8], mybir.dt.uint32, tag="tix")
            nc.vector.max(out=top[:], in_=lt[:])
            nc.vector.max_index(out=tix[:], in_max=top[:], in_values=lt[:])
            nc.vector.tensor_copy(i1b[:, t:t + 1], tix[:, 0:1])
            nc.vector.tensor_copy(i2b[:, t:t + 1], tix[:, 1:2])

            dif = wk.tile([P, 1], F32, tag="dif")
            nc.vector.tensor_tensor(out=dif[:], in0=top[:, 0:1], in1=top[:, 1:2], op=mybir.AluOpType.subtract)
            nc.scalar.activation(w1b[:, t:t + 1], dif[:], mybir.ActivationFunctionType.Sigmoid)
            nc.scalar.activation(w2b[:, t:t + 1], dif[:], mybir.ActivationFunctionType.Sigmoid, scale=-1.0)

            eq1 = eq1b[:, t * E:(t + 1) * E]
            eq2 = eq2b[:, t * E:(t + 1) * E]
            msk = maskb[:, t * E:(t + 1) * E]
            nc.vector.tensor_scalar(eq1, iota16f[:], i1b[:, t:t + 1], None, op0=mybir.AluOpType.is_equal)
            nc.vector.tensor_scalar(eq2, iota16f[:], i2b[:, t:t + 1], None, op0=mybir.AluOpType.is_equal)
            nc.vector.tensor_tensor(out=msk, in0=eq1, in1=eq2, op=mybir.AluOpType.add)

            csp = ps.tile([16, 1], F32, tag="ps")
            nc.tensor.matmul(csp[:], msk, onesP[:], start=True, stop=True)
            nc.vector.tensor_copy(colsT[:, t:t + 1], csp[:])

            # softmax scores for aux loss
            mx = wk.tile([P, 1], F32, tag="mx")
            nc.vector.reduce_max(mx[:], lt[:], axis=mybir.AxisListType.X)
            nmx = wk.tile([P, 1], F32, tag="nmx")
            nc.vector.tensor_scalar_mul(nmx[:], mx[:], -1.0)
            ex = wk.tile([P, E], F32, tag="ex")
            nc.scalar.activation(ex[:], lt[:], mybir.ActivationFunctionType.Exp, bias=nmx[:])
            sm = wk.tile([P, 1], F32, tag="sm")
            nc.vector.reduce_sum(sm[:], ex[:], axis=mybir.AxisListType.X)
            rs = wk.tile([P, 1], F32, tag="rs")
            nc.vector.reciprocal(rs[:], sm[:])
            sc = wk.tile([P, E], F32, tag="sc")
            nc.vector.tensor_scalar(sc[:], ex[:], rs[:], None, op0=mybir.AluOpType.mult)
            nc.vector.tensor_tensor(out=sacc[:], in0=sacc[:], in1=sc[:], op=mybir.AluOpType.add)

        # ---------- capacity offsets: exclusive scan over tiles ----------
        incl = big.tile([16, NT], F32)
        zer = wk.tile([16, NT], F32, tag="zer")
        nc.vector.memset(zer[:], 0.0)
        nc.vector.tensor_tensor_scan(incl[:], colsT[:], zer[:], 0.0,
                                     op0=mybir.AluOpType.add, op1=mybir.AluOpType.add)
        excl = big.tile([16, NT], F32)
        nc.vector.tensor_tensor(out=excl[:], in0=incl[:], in1=colsT[:], op=mybir.AluOpType.subtract)
        nc.sync.dma_start(out=excl_d[:], in_=excl[:])

        # ---------- positions + dispatch records ----------
        pos1 = big.tile([P, NT], F32)
        pos2 = big.tile([P, NT], F32)
        for t in range(NT):
            otile = wk.tile([1, E], F32, tag="otile", name="otile")
            nc.sync.dma_start(out=otile[:], in_=excl_d[:, t:t + 1].rearrange("a b -> b a"))
            rp = ps.tile([P, E], F32, tag="ps")
            nc.tensor.matmul(rp[:], sut[:], maskb[:, t * E:(t + 1) * E], start=True, stop=False)
            nc.tensor.matmul(rp[:], ones1[:], otile[:], start=False, stop=True)
            for (eqb, ib, posb) in ((eq1b, i1b, pos1), (eq2b, i2b, pos2)):
                tmp = wk.tile([P, E], F32, tag="tmp")
                nc.vector.tensor_tensor(out=tmp[:], in0=rp[:], in1=eqb[:, t * E:(t + 1) * E], op=mybir.AluOpType.mult)
                slot = wk.tile([P, 1], F32, tag="slot")
                nc.vector.reduce_sum(slot[:], tmp[:], axis=mybir.AxisListType.X)
                nc.vector.tensor_scalar(posb[:, t:t + 1], ib[:, t:t + 1], float(C), slot[:],
                                        op0=mybir.AluOpType.mult, op1=mybir.AluOpType.add)

        posI = big.tile([P, NT, 2], I32)
        nc.vector.tensor_copy(posI[:, :, 0], pos1[:])
        nc.vector.tensor_copy(posI[:, :, 1], pos2[:])
        nc.sync.dma_start(out=pos_d[:].rearrange("(t p) k -> p t k", p=P), in_=posI[:])

        # local offsets; negatives -> huge so bounds check drops them
        locs = []
        for _ki, posb in enumerate((pos1, pos2)):
            loc = big.tile([P, NT], F32, name=f"loc{_ki}", tag=f"loc{_ki}")
            nc.vector.tensor_scalar(loc[:], posb[:], ebc[:], None, op0=mybir.AluOpType.subtract)
            neg = wk.tile([P, NT], F32, tag="neg")
            nc.vector.tensor_scalar(neg[:], loc[:], 0.0, 1.0e6, op0=mybir.AluOpType.is_lt, op1=mybir.AluOpType.mult)
            nc.vector.tensor_tensor(out=loc[:], in0=loc[:], in1=neg[:], op=mybir.AluOpType.add)
            locI = big.tile([P, NT], I32, name=f"locI{_ki}", tag=f"locI{_ki}")
            nc.vector.tensor_copy(locI[:], loc[:])
            locs.append(locI)

        recall = big.tile([P, NT, 2, 2], F32)   # [p, t, k, (id, w)]
        for ki, wb in ((0, w1b), (1, w2b)):
            nc.vector.tensor_copy(recall[:, :, ki, 0], tokidf[:])
            nc.vector.tensor_copy(recall[:, :, ki, 1], wb[:])
        for t in range(NT):
            for ki, locI in enumerate(locs):
                nc.gpsimd.indirect_dma_start(
                    out=idw_d[:], out_offset=bass.IndirectOffsetOnAxis(ap=locI[:, t:t + 1], axis=0),
                    in_=recall[:, t, ki, :], in_offset=None, bounds_check=SLOTS - 1, oob_is_err=False)

        # ---------- aux loss ----------
        pf = ps.tile([16, 1], F32, tag="ps")
        nc.tensor.matmul(pf[:], sacc[:], onesP[:], start=True, stop=True)
        pfs = wk.tile([16, 1], F32, tag="pfs")
        nc.vector.tensor_copy(pfs[:], pf[:])
        prod = wk.tile([16, 1], F32, tag="prod")
        nc.vector.tensor_tensor(out=prod[:], in0=pfs[:], in1=incl[:, NT - 1:NT], op=mybir.AluOpType.mult)
        axp = ps.tile([1, 1], F32, tag="ps")
        nc.tensor.matmul(axp[:], prod[:], onesP[0:16, 0:1], start=True, stop=True)
        axs = wk.tile([1, 1], F32, tag="axs")
        nc.vector.tensor_scalar_mul(axs[:], axp[:], float(E) / (N * N))
        nc.sync.dma_start(out=aux_d[:], in_=axs[:])

        tc.no_sync_barrier()

        # ---------- shared expert on this core's token shard ----------
        xs = [wp.tile([P, TS], F32R, tag=f"xs{k}", name=f"xs{k}") for k in range(KD)]
        for k in range(KD):
            nc.sync.dma_start(out=xs[k][:], in_=xts_d[k * P:(k + 1) * P, :])
        hs = [wp.tile([P, TS], F32R, tag=f"hs{i}", name=f"hs{i}") for i in range(KI)]
        for i in range(KI):
            gp = ps.tile([P, TS], F32, tag="gpup", bufs=2, name="gp")
            up = ps.tile([P, TS], F32, tag="gpup", bufs=2, name="up")
            for k in range(KD):
                nc.tensor.matmul(gp[:], sgT[k][:, i * P:(i + 1) * P], xs[k][:],
                                 start=(k == 0), stop=(k == KD - 1))
            for k in range(KD):
                nc.tensor.matmul(up[:], suT[k][:, i * P:(i + 1) * P], xs[k][:],
                                 start=(k == 0), stop=(k == KD - 1))
            sg = wk.tile([P, TS], F32, tag="sgs")
            nc.scalar.activation(sg[:], gp[:], mybir.ActivationFunctionType.Sigmoid)
            sil = wk.tile([P, TS], F32, tag="sils")
            nc.vector.tensor_tensor(out=sil[:], in0=sg[:], in1=gp[:], op=mybir.AluOpType.mult)
            nc.vector.tensor_tensor(out=hs[i][:], in0=sil[:], in1=up[:], op=mybir.AluOpType.mult)
        for sl in range(TS // P):
            shs = wk.tile([P, D], F32, tag="shs")
            for nh in range(2):
                sp = ps.tile([P, I], F32, tag="ps")
                for i in range(KI):
                    nc.tensor.matmul(sp[:], hs[i][:, sl * P:(sl + 1) * P], sdT[i][:, nh * I:(nh + 1) * I],
                                     start=(i == 0), stop=(i == KI - 1))
                nc.vector.tensor_copy(shs[:, nh * I:(nh + 1) * I], sp[:])
            nc.sync.dma_start(out=sh_d[sl * P:(sl + 1) * P, :], in_=shs[:])


        tc.no_sync_barrier()

        # ---------- dispatch gather + transpose ----------
        XT = [[wp.tile([P, C], F32R, tag=f"XT{e}_{k}", name=f"XT{e}_{k}") for k in range(KD)] for e in range(EPC)]
        wslot = [wp.tile([P, 1], F32, tag=f"ws{s}", name=f"ws{s}") for s in range(NS)]
        for s in range(NS):
            e, sl = s // KI, s % KI
            idw = wk.tile([P, 2], F32, tag="idw")
            nc.sync.dma_start(out=idw[:], in_=idw_d[s * P:(s + 1) * P, :])
            nc.vector.tensor_copy(wslot[s][:], idw[:, 1:2])
            idsi = wk.tile([P, 1], I32, tag="idsi")
            nc.vector.tensor_copy(idsi[:], idw[:, 0:1])
            xg = wk.tile([P, D], F32, tag="xg")
            nc.gpsimd.indirect_dma_start(
                out=xg[:], out_offset=None, in_=x_d[:],
                in_offset=bass.IndirectOffsetOnAxis(ap=idsi[:, :1], axis=0),
                bounds_check=N - 1, oob_is_err=False)
            for k in range(KD):
                tp = ps.tile([P, P], F32, tag="tp", bufs=2)
                nc.tensor.transpose(out=tp[:], in_=xg[:, k * P:(k + 1) * P], identity=ident[:])
                nc.vector.tensor_copy(XT[e][k][:, sl * P:(sl + 1) * P], tp[:])

        tc.no_sync_barrier()

        # ---------- routed experts FFN (fp32r) ----------
        for e in range(EPC):
            hh = [wp.tile([P, C], F32R, tag=f"hh{e}_{i}", name=f"hh{e}_{i}") for i in range(KI)]
            for i in range(KI):
                gp = ps.tile([P, C], F32, tag="gpup", bufs=2, name="gp")
                up = ps.tile([P, C], F32, tag="gpup", bufs=2, name="up")
                for k in range(KD):
                    nc.tensor.matmul(gp[:], rgT[e][k][:, i * P:(i + 1) * P], XT[e][k][:],
                                     start=(k == 0), stop=(k == KD - 1))
                for k in range(KD):
                    nc.tensor.matmul(up[:], ruT[e][k][:, i * P:(i + 1) * P], XT[e][k][:],
                                     start=(k == 0), stop=(k == KD - 1))
                sg = wk.tile([P, C], F32, tag="sg")
                nc.scalar.activation(sg[:], gp[:], mybir.ActivationFunctionType.Sigmoid)
                sil = wk.tile([P, C], F32, tag="sil")
                nc.vector.tensor_tensor(out=sil[:], in0=sg[:], in1=gp[:], op=mybir.AluOpType.mult)
                nc.vector.tensor_tensor(out=hh[i][:], in0=sil[:], in1=up[:], op=mybir.AluOpType.mult)
            for sl in range(KI):
                ysb = wk.tile([P, D], F32, tag="ysb")
                for nh in range(2):
                    yp = ps.tile([P, I], F32, tag="ps")
                    for i in range(KI):
                        nc.tensor.matmul(yp[:], hh[i][:, sl * P:(sl + 1) * P], rdT[e][i][:, nh * I:(nh + 1) * I],
                                         start=(i == 0), stop=(i == KI - 1))
                    nc.vector.tensor_scalar(ysb[:, nh * I:(nh + 1) * I], yp[:], wslot[e * KI + sl][:], None,
                                            op0=mybir.AluOpType.mult)
                nc.sync.dma_start(out=y_d[(e * KI + sl) * P:(e * KI + sl + 1) * P, :], in_=ysb[:])

    nc.compile()
    return nc


def kernel(x, gate_w, sg_w, su_w, sd_w, rg_w, ru_w, rd_w):
    x = np.ascontiguousarray(np.asarray(x, dtype=np.float32))
    flat = x.reshape(N, D)
    xT = np.ascontiguousarray(flat.T)
    xh = np.ascontiguousarray(xT.astype(ml_dtypes.bfloat16).astype(np.float32))
    xr = np.ascontiguousarray(xT - xh)
    gwT = np.ascontiguousarray(np.asarray(gate_w, np.float32).T)   # (D, E)
    gh, gm, gl = _split3(gwT)
    gparts = (gh, gm, gl)
    gws = np.concatenate([np.asarray(gparts[gi], dtype=np.float32) for (_, gi) in PASSES], axis=0)

    sgT = np.ascontiguousarray(np.asarray(sg_w, np.float32).T)
    suT = np.ascontiguousarray(np.asarray(su_w, np.float32).T)
    sdT = np.ascontiguousarray(np.asarray(sd_w, np.float32).T)
    rgT = np.ascontiguousarray(np.asarray(rg_w, np.float32).transpose(0, 2, 1))
    ruT = np.ascontiguousarray(np.asarray(ru_w, np.float32).transpose(0, 2, 1))
    rdT = np.ascontiguousarray(np.asarray(rd_w, np.float32).transpose(0, 2, 1))

    if "nc" not in _cache:
        _cache["nc"] = _build()
    nc = _cache["nc"]

    in_maps = []
    for c in range(NCORES):
        in_maps.append({
            "x": flat,
            "xph": xh, "xpm": xr,
            "gws": gws,
            "xts": np.ascontiguousarray(xT[:, c * TS:(c + 1) * TS]),
            "sgT": sgT, "suT": suT, "sdT": sdT,
            "rgT": rgT[c * EPC:(c + 1) * EPC],
            "ruT": ruT[c * EPC:(c + 1) * EPC],
            "rdT": rdT[c * EPC:(c + 1) * EPC],
            "eoff": np.full((1, 1), c * EPC * C, np.float32),
        })
    res = run_bass_kernel_spmd(nc, in_maps, core_ids=list(range(NCORES))).results

    y_all = np.concatenate([r["y"] for r in res], axis=0)          # (E*C, D)
    sh_all = np.concatenate([r["sh"] for r in res], axis=0)        # (N, D)
    pos = res[0]["pos"]                                            # (N, 2)
    out = sh_all + y_all[pos[:, 0]] + y_all[pos[:, 1]]
    aux = np.float32(res[0]["aux"][0, 0])
    return out.reshape(B, T, D), np.asarray(aux, dtype=np.float32)
